# revision 35
# baseline (speedup 1.0000x reference)
"""Trainium2 Bass kernel for multi-head causal self-attention.

Problem: X [4, 2048, 1024] fp32, Wq/Wk/Wv/Wo [1024, 1024], H=16 heads, HD=64.
reference: out = softmax_causal((X@Wq) (X@Wk)^T / 8) (X@Wv) merged @ Wo.

Sharding over 8 NeuronCores: core c handles batch b = c // 2 and head group
hg = c % 2 (8 heads each). Each core computes a partial [2048, 1024] output
(its heads' contribution through Wo's row shard); the host sums the two
partials per batch (the tensor-parallel all-reduce, done during unsharding).

Per-core dataflow (bf16 operands, fp32 PSUM accumulation), software-pipelined
so the scalar engine's exp stream starts early and runs continuously while
the PE fills gaps with projection / output-projection matmuls:

  ramp:   X arrives pre-transposed from the host (the on-chip DMA XBAR
          transpose is a serialized ~40us unit); DMAs ordered by
          criticality (X^T rows + Wq + Wk gate the first attention
          groups; the ramp is HBM-bandwidth-bound at ~360GB/s/core).
          Q^T/K^T for q-chunk 0 accumulate per-d-chunk as X^T rows land
          (8 PSUM banks), V rows 0..511 right after.
  stage j (= q-chunk): attention for q-chunk j; interleaved filler work =
          Q^T/K^T chunk j+1, V rows for stage j+1; all Wo chains are
          deferred to stage 3 whose long exp stream (ACT-bound) hides
          them. Fillers pause near each pc boundary so the normalize
          chain is not queued behind them on the DVE.
  group (j, pc, i): one 128-wide k-chunk for one head pair pc:
      S^T pair [128k, 2x512q] -> one 2-bank PSUM group (heads row-packed
      in the PE at tile_position 64h, streaming concurrently)
      one exp ACTIVATE over the [128, 1024] group -> et bf16 (batching
      amortizes ACT's 352-cycle per-instruction overhead)
      causal diagonal: post-exp gpsimd affine_select zeroes the upper
      triangle of the diagonal 128-block in et (SBUF; gpsimd cannot
      touch PSUM). Fully-masked leading columns are simply never read.
      AV: av_h += vt_h.T @ et_h; vt col layout gives h0 output at PSUM
      partitions 0..63 + denominator row 64, h1 denominator at row 0 +
      output partitions 64..127 (all row ranges 0/64-based: engine APs
      need 32-aligned partition bases).
  normalize (j, pc): DVE-copy av -> SBUF, gather the two denominator
      rows to partitions 0..1 via two small DMAs, cast bf16, broadcast
      to [128,512] with one PE selector matmul (rsel.T @ dens) -- NOT
      gpsimd partition_broadcast, whose nonzero output base silently
      miswrites on HW -- then one reciprocal_approx_fast and two DVE
      multiplies -> ot[pc] bf16.
  Wo: out rows st -> sum over pc of ot[pc].T @ Wo chunk, DVE copy, DMA
      out (tail chunks split across the sync+scalar DMA queues).

Measured: 332us HW exec, rel err 3.8e-3 (baseline 440us).
"""

import sys
from collections import deque

for _p in ("/opt/trn_rl_repo", "/root/.axon_site/_ro/trn_rl_repo"):
    if _p not in sys.path:
        sys.path.insert(0, _p)

import ml_dtypes
import numpy as np

import concourse.bass as bass
import concourse.mybir as mybir
import concourse.tile as tile
from concourse import bacc
from concourse.bass_utils import run_bass_kernel_spmd

F32 = mybir.dt.float32
BF16 = mybir.dt.bfloat16
EXPF = mybir.ActivationFunctionType.Exp

B, S, D, H = 4, 2048, 1024, 16
HD = D // H           # 64
HL = H // 2           # 8 heads per core
DL = HL * HD          # 512 local proj width
NEG = -30000.0        # causal mask additive value (exp underflows to 0)

N_DC = D // 128       # 8  d-chunks (projection contraction)
N_PC = HL // 2        # 4  head pairs
N_Q = S // 512        # 4  q-chunks (= stages)
N_ST = S // 128       # 16 s-tiles (output rows / V rows)
N_CC = D // 512       # 2  out column chunks


def build_program():
    s, d = S, D

    nc = bacc.Bacc("TRN2", target_bir_lowering=False, debug=False)

    # X arrives pre-transposed from the host ([d, s]): the on-chip DMA
    # XBAR transpose is a single serialized unit (~40us for 4MB); plain
    # row loads of X^T stream at full DMA bandwidth instead.
    X = nc.dram_tensor("X", [d, s], BF16, kind="ExternalInput")
    WQ = nc.dram_tensor("WQ", [d, DL], BF16, kind="ExternalInput")
    WK = nc.dram_tensor("WK", [d, DL], BF16, kind="ExternalInput")
    WV = nc.dram_tensor("WV", [d, DL], BF16, kind="ExternalInput")
    WO = nc.dram_tensor("WO", [DL, d], BF16, kind="ExternalInput")
    OUT = nc.dram_tensor("OUT", [s, d], F32, kind="ExternalOutput")

    with tile.TileContext(nc) as tc:
        with tc.tile_pool(name="persist", bufs=1) as persist:
            qt = [persist.tile([128, s], BF16, name=f"qt{i}") for i in range(N_PC)]
            kt = [persist.tile([128, s], BF16, name=f"kt{i}") for i in range(N_PC)]
            ot = [persist.tile([128, s], BF16, name=f"ot{i}") for i in range(N_PC)]
            xt = [persist.tile([128, s], BF16, name=f"xt{i}") for i in range(N_DC)]
            # AV stationary operand, per s-tile, per head block of 128 cols:
            #   even head (h0): cols 0:64 = V, col 64 = ones (denominator row)
            #   odd head (h1):  col 0 = ones, cols 64:128 = V
            # (the remaining cols feed PSUM rows nothing ever reads; all
            # consumed row ranges start at partition 0 or 64 — the DVE/BIR
            # partition-alignment rule)
            vt = [persist.tile([128, HL, 128], BF16, name=f"vt{i}")
                  for i in range(N_ST)]
            wq = persist.tile([128, N_DC, DL], BF16, name="wq")
            wk = persist.tile([128, N_DC, DL], BF16, name="wk")
            wv = persist.tile([128, N_DC, DL], BF16, name="wv")
            wo = persist.tile([128, N_PC, d], BF16, name="wo")

            # ---- input DMAs. The ramp is HBM-bandwidth-bound (~9MB of
            # inputs at ~360GB/s/core), so order by criticality: X^T rows
            # and Wq/Wk gate the first attention groups; Wv is needed a few
            # us later, Wo only tens of us in. ----
            nc.scalar.dma_start(
                wq[:], WQ.ap().rearrange("(c p) m -> p c m", p=128))
            for dc in range(0, 4):
                nc.sync.dma_start(
                    xt[dc][:], X[dc * 128:(dc + 1) * 128, :])
            nc.scalar.dma_start(
                wk[:], WK.ap().rearrange("(c p) m -> p c m", p=128))
            for dc in range(4, 8):
                nc.scalar.dma_start(
                    xt[dc][:], X[dc * 128:(dc + 1) * 128, :])
            nc.sync.dma_start(
                wv[:], WV.ap().rearrange("(c p) m -> p c m", p=128))
            nc.scalar.dma_start(
                wo[:], WO.ap().rearrange("(c p) m -> p c m", p=128))

            # ---- ramp: Q^T/K^T for q-chunk 0, accumulated per d-chunk so
            # matmuls start as soon as each X^T chunk lands (all Q chains
            # first: K is gated by the later wk DMA) ----
            with tc.tile_pool(name="ramp", bufs=1, space="PSUM") as ramp:
                accs = {}
                for wi in range(2):
                    for pc in range(N_PC):
                        accs[wi, pc] = ramp.tile(
                            [128, 512], F32, tag=f"acc{wi}_{pc}",
                            name=f"acc{wi}_{pc}")
                for wi, wsb in ((0, wq), (1, wk)):
                    for dc in range(N_DC):
                        for pc in range(N_PC):
                            nc.tensor.matmul(
                                accs[wi, pc][:],
                                wsb[:, dc, pc * 128:(pc + 1) * 128],
                                xt[dc][:, 0:512],
                                start=(dc == 0), stop=(dc == N_DC - 1))
                for wi, dst in ((0, qt), (1, kt)):
                    for pc in range(N_PC):
                        nc.scalar.copy(
                            dst[pc][:, 0:512], accs[wi, pc][:])

            # ones columns for the AV denominator rows (gpsimd, overlaps
            # the ramp DMAs/matmuls; only needed once AV starts)
            for st in range(N_ST):
                for g in range(N_PC):
                    nc.gpsimd.memset(vt[st][:, 2 * g, 64:65], 1.0)
                    nc.gpsimd.memset(vt[st][:, 2 * g + 1, 0:1], 1.0)
            # denominator-broadcast selector: rsel[c, p] = 1 iff p//64 == c,
            # so rsel.T @ [den0; den1] fills PSUM rows 0:64 with den0 and
            # 64:128 with den1
            rsel = persist.tile([2, 128], BF16)
            nc.gpsimd.memset(rsel[:], 1.0)
            nc.gpsimd.affine_select(
                out=rsel[:], in_=rsel[:],
                compare_op=mybir.AluOpType.is_ge, fill=0.0,
                base=0, pattern=[[1, 128]], channel_multiplier=-64,
            )
            nc.gpsimd.affine_select(
                out=rsel[:], in_=rsel[:],
                compare_op=mybir.AluOpType.is_ge, fill=0.0,
                base=63, pattern=[[-1, 128]], channel_multiplier=64,
            )


            with (
                tc.tile_pool(name="spp", bufs=2, space="PSUM") as spp,
                tc.tile_pool(name="avp", bufs=1, space="PSUM") as avp,
                tc.tile_pool(name="outp", bufs=2, space="PSUM") as outp,
                tc.tile_pool(name="etp", bufs=3) as etp,
                tc.tile_pool(name="nrm", bufs=2) as nrm,
            ):
                fillers = deque()
                uid = [0]

                def nid():
                    uid[0] += 1
                    return uid[0]

                def qk_chain_ops(wsb, dst, pc, nq):
                    box = {}
                    ops = []
                    for dc in range(N_DC):
                        def mm(dc=dc, box=box, wsb=wsb, pc=pc, nq=nq):
                            if dc == 0:
                                box["ps"] = outp.tile(
                                    [128, 512], F32, tag="outp",
                                    name=f"qkps{nid()}")
                            nc.tensor.matmul(
                                box["ps"][:],
                                wsb[:, dc, pc * 128:(pc + 1) * 128],
                                xt[dc][:, nq * 512:(nq + 1) * 512],
                                start=(dc == 0), stop=(dc == N_DC - 1))
                        ops.append(mm)

                    def cp(box=box, dst=dst, pc=pc, nq=nq):
                        # scalar engine: idle in the projection-heavy stages,
                        # and these copies gate the next stage's exps anyway
                        nc.scalar.copy(
                            dst[pc][:, nq * 512:(nq + 1) * 512], box["ps"][:])
                    ops.append(cp)
                    return ops

                def v_chain_ops(st):
                    box = {}
                    ops = []
                    for dc in range(N_DC):
                        def mm(dc=dc, box=box, st=st):
                            if dc == 0:
                                box["ps"] = outp.tile(
                                    [128, 512], F32, tag="outp",
                                    name=f"vps{nid()}")
                            nc.tensor.matmul(
                                box["ps"][:],
                                xt[dc][:, st * 128:(st + 1) * 128],
                                wv[:, dc, :],
                                start=(dc == 0), stop=(dc == N_DC - 1))
                        ops.append(mm)

                    def cp(box=box, st=st):
                        ps4 = box["ps"][:].rearrange(
                            "p (g t e) -> p g t e", t=2, e=64)
                        vtr = vt[st].rearrange("p (g t) c -> p g t c", t=2)
                        nc.scalar.copy(
                            vtr[:, :, 0:1, 0:64], ps4[:, :, 0:1, :])
                        nc.scalar.copy(
                            vtr[:, :, 1:2, 64:128], ps4[:, :, 1:2, :])
                    ops.append(cp)
                    return ops

                def wo_chain_ops(st, cc, tail=False):
                    box = {}
                    ops = []
                    for pc in range(N_PC):
                        def mm(pc=pc, box=box, st=st, cc=cc):
                            if pc == 0:
                                box["ps"] = outp.tile(
                                    [128, 512], F32, tag="outp",
                                    name=f"wops{nid()}")
                            nc.tensor.matmul(
                                box["ps"][:],
                                ot[pc][:, st * 128:(st + 1) * 128],
                                wo[:, pc, cc * 512:(cc + 1) * 512],
                                start=(pc == 0), stop=(pc == N_PC - 1))
                        ops.append(mm)

                    def fin(box=box, st=st, cc=cc, tail=tail):
                        osb = nrm.tile(
                            [128, 512], F32, tag="osb", name=f"osb{nid()}")
                        nc.vector.tensor_copy(osb[:], box["ps"][:])
                        # past the last exp the scalar hwdge queue is free
                        eng = nc.scalar if (tail and cc == 1) else nc.sync
                        eng.dma_start(
                            OUT[st * 128:(st + 1) * 128,
                                cc * 512:(cc + 1) * 512], osb[:])
                    ops.append(fin)
                    return ops

                def normalize(j, pc, av):
                    js = slice(j * 512, (j + 1) * 512)
                    orwA = nrm.tile([128, 512], F32, tag="orwA",
                                    name=f"orwA{nid()}")
                    orwB = nrm.tile([128, 512], F32, tag="orwB",
                                    name=f"orwB{nid()}")
                    # full-tile copies: rows 65+ of av0 / 1..63 of av1 are
                    # never read, copying them is cheaper than extra ops
                    nc.vector.tensor_copy(orwA[:], av[0][:])
                    nc.vector.tensor_copy(orwB[:], av[1][:])
                    dd = nrm.tile([2, 512], F32, tag="dd", name=f"dd{nid()}")
                    nc.sync.dma_start(dd[0:1, :], orwA[64:65, :])
                    nc.sync.dma_start(dd[1:2, :], orwB[0:1, :])
                    ddb = nrm.tile([2, 512], BF16, tag="ddb",
                                   name=f"ddb{nid()}")
                    nc.vector.tensor_copy(ddb[:], dd[:])
                    bcp = outp.tile([128, 512], F32, tag="outp",
                                    name=f"bcp{nid()}")
                    nc.tensor.matmul(bcp[:], rsel[:], ddb[:],
                                     start=True, stop=True)
                    bcr = nrm.tile([128, 512], F32, tag="bcr",
                                   name=f"bcr{nid()}")
                    nc.vector.reciprocal_approx_fast(bcr[:], bcp[:])
                    nc.vector.tensor_mul(
                        ot[pc][0:64, js], orwA[0:64, :], bcr[0:64, :])
                    nc.vector.tensor_mul(
                        ot[pc][64:128, js], orwB[64:128, :], bcr[64:128, :])

                def attention_stage(j):
                    n_i = 4 * j + 4
                    # filler work to interleave into this stage: projections
                    # for the next stage; ALL Wo chains deferred to stage 3,
                    # whose exp stream is long enough to hide them
                    if j + 1 < N_Q:
                        for wsb, dst in ((wq, qt), (wk, kt)):
                            for pc in range(N_PC):
                                fillers.extend(
                                    qk_chain_ops(wsb, dst, pc, j + 1))
                        for st in range(4 * (j + 1), 4 * (j + 1) + 4):
                            fillers.extend(v_chain_ops(st))
                    if j == 3:
                        for st in range(0, 12):
                            for cc in range(N_CC):
                                fillers.extend(wo_chain_ops(st, cc))
                    n_groups = N_PC * n_i
                    pace = max(1, -(-len(fillers) // n_groups))

                    for pc in range(N_PC):
                        av = [avp.tile([128, 512], F32, tag=f"av{h}",
                                       name=f"av{j}_{pc}_{h}")
                              for h in (0, 1)]

                        def issue_av(i, rs, et, pc=pc, av=av, n_i=n_i):
                            for h in (0, 1):
                                nc.tensor.matmul(
                                    av[h][:, rs:512],
                                    vt[i][:, 2 * pc + h, :],
                                    et[:, 512 * h + rs:512 * h + 512],
                                    start=(i == 0), stop=(i == n_i - 1))

                        prev = None
                        for i in range(n_i):
                            r = i - 4 * j
                            rs = max(r, 0) * 128
                            stp = spp.tile([128, 1024], F32, tag="stp",
                                           name=f"stp{j}_{pc}_{i}")
                            for h in (0, 1):
                                hs = slice(64 * h, 64 * h + 64)
                                nc.tensor.matmul(
                                    stp[:, 512 * h + rs:512 * h + 512],
                                    kt[pc][hs, i * 128:(i + 1) * 128],
                                    qt[pc][hs, j * 512 + rs:(j + 1) * 512],
                                    start=True, stop=True,
                                    tile_position=(64 * h, 0))
                            if prev is not None:
                                issue_av(*prev)
                            et = etp.tile([128, 1024], BF16, tag="et",
                                          name=f"et{j}_{pc}_{i}")
                            if rs >= 256:
                                # mostly-masked diagonal group: two narrow
                                # exps beat one wide one (352-cycle fixed
                                # cost per ACTIVATE)
                                for h in (0, 1):
                                    sl = slice(512 * h + rs, 512 * h + 512)
                                    nc.scalar.activation(
                                        et[:, sl], stp[:, sl], EXPF,
                                        scale=0.125)
                            else:
                                nc.scalar.activation(
                                    et[:], stp[:], EXPF, scale=0.125)
                            if r >= 0:
                                # causal mask: zero the strict upper triangle
                                # of the diagonal 128-block (keep q >= k)
                                for h in (0, 1):
                                    sl = slice(512 * h + rs, 512 * h + rs + 128)
                                    nc.gpsimd.affine_select(
                                        out=et[:, sl], in_=et[:, sl],
                                        compare_op=mybir.AluOpType.is_ge,
                                        fill=0.0, base=0, pattern=[[1, 128]],
                                        channel_multiplier=-1)
                            prev = (i, rs, et)
                            # hold fillers near the pc boundary so the
                            # normalize's DVE/DMA chain isn't queued behind
                            # them (the next pc's AV reuses the av banks)
                            if i < n_i - 2:
                                for _ in range(pace):
                                    if fillers:
                                        fillers.popleft()()
                        issue_av(*prev)
                        normalize(j, pc, av)
                        for _ in range(2 * pace):
                            if fillers:
                                fillers.popleft()()
                    while fillers:
                        fillers.popleft()()

                # ---- V rows for stage 0, then the staged attention ----
                for st in range(4):
                    for op in v_chain_ops(st):
                        op()
                for j in range(N_Q):
                    attention_stage(j)
                # ---- tail: output rows of the last stage ----
                for st in range(12, 16):
                    for cc in range(N_CC):
                        for op in wo_chain_ops(st, cc, tail=True):
                            op()

    nc.compile()
    return nc


_NC_CACHE = {}


def _get_program():
    key = (S, D, HL)
    if key not in _NC_CACHE:
        _NC_CACHE[key] = build_program()
    return _NC_CACHE[key]


def _bf16(a):
    return np.ascontiguousarray(a.astype(ml_dtypes.bfloat16))


def make_in_maps(X, Wq, Wk, Wv, Wo):
    in_maps = []
    for c in range(8):
        b, hg = c // 2, c % 2
        cs = slice(hg * DL, hg * DL + DL)
        in_maps.append({
            "X": _bf16(X[b].T),
            "WQ": _bf16(Wq[:, cs]),
            "WK": _bf16(Wk[:, cs]),
            "WV": _bf16(Wv[:, cs]),
            "WO": _bf16(Wo[cs, :]),
        })
    return in_maps


def gather_out(results):
    out = np.empty((B, S, D), dtype=np.float32)
    for b in range(B):
        out[b] = results[2 * b]["OUT"] + results[2 * b + 1]["OUT"]
    return out


def kernel(X, Wq, Wk, Wv, Wo):
    X = np.asarray(X, dtype=np.float32)
    Wq = np.asarray(Wq, dtype=np.float32)
    Wk = np.asarray(Wk, dtype=np.float32)
    Wv = np.asarray(Wv, dtype=np.float32)
    Wo = np.asarray(Wo, dtype=np.float32)

    nc = _get_program()
    in_maps = make_in_maps(X, Wq, Wk, Wv, Wo)
    res = run_bass_kernel_spmd(nc, in_maps, list(range(8)), trace=False)
    return gather_out(res.results)


if __name__ == "__main__":
    rng = np.random.default_rng(0)
    scale = 1.0 / np.sqrt(D)
    inputs = {
        "X": rng.standard_normal((B, S, D), dtype=np.float32),
        "Wq": rng.standard_normal((D, D), dtype=np.float32) * scale,
        "Wk": rng.standard_normal((D, D), dtype=np.float32) * scale,
        "Wv": rng.standard_normal((D, D), dtype=np.float32) * scale,
        "Wo": rng.standard_normal((D, D), dtype=np.float32) * scale,
    }
    out = kernel(**inputs)
    print("kernel output shape:", out.shape)


# revision 37
# speedup vs baseline: 1.0052x; 1.0052x over previous
"""Trainium2 Bass kernel for multi-head causal self-attention.

Problem: X [4, 2048, 1024] fp32, Wq/Wk/Wv/Wo [1024, 1024], H=16 heads, HD=64.
reference: out = softmax_causal((X@Wq) (X@Wk)^T / 8) (X@Wv) merged @ Wo.

Sharding over 8 NeuronCores: core c handles batch b = c // 2 and head group
hg = c % 2 (8 heads each). Each core computes a partial [2048, 1024] output
(its heads' contribution through Wo's row shard); the host sums the two
partials per batch (the tensor-parallel all-reduce, done during unsharding).

Per-core dataflow (bf16 operands, fp32 PSUM accumulation), software-pipelined
so the scalar engine's exp stream starts early and runs continuously while
the PE fills gaps with projection / output-projection matmuls:

  ramp:   X arrives pre-transposed from the host (the on-chip DMA XBAR
          transpose is a serialized ~40us unit); DMAs ordered by
          criticality (X^T rows + Wq + Wk gate the first attention
          groups; the ramp is HBM-bandwidth-bound at ~360GB/s/core).
          Q^T/K^T for q-chunk 0 accumulate per-d-chunk as X^T rows land
          (8 PSUM banks), V rows 0..511 right after.
  stage j (= q-chunk): attention for q-chunk j; interleaved filler work =
          Q^T/K^T chunk j+1, V rows for stage j+1; all Wo chains are
          deferred to stage 3 whose long exp stream (ACT-bound) hides
          them. Fillers pause near each pc boundary so the normalize
          chain is not queued behind them on the DVE.
  group (j, pc, i): one 128-wide k-chunk for one head pair pc:
      S^T pair [128k, 2x512q] -> one 2-bank PSUM group (heads row-packed
      in the PE at tile_position 64h, streaming concurrently)
      one exp ACTIVATE over the [128, 1024] group -> et bf16 (batching
      amortizes ACT's 352-cycle per-instruction overhead)
      causal diagonal: post-exp gpsimd affine_select zeroes the upper
      triangle of the diagonal 128-block in et (SBUF; gpsimd cannot
      touch PSUM). Fully-masked leading columns are simply never read.
      AV: av_h += vt_h.T @ et_h; vt col layout gives h0 output at PSUM
      partitions 0..63 + denominator row 64, h1 denominator at row 0 +
      output partitions 64..127 (all row ranges 0/64-based: engine APs
      need 32-aligned partition bases).
  normalize (j, pc): DVE-copy av -> SBUF, gather the two denominator
      rows to partitions 0..1 via two small DMAs, cast bf16, broadcast
      to [128,512] with one PE selector matmul (rsel.T @ dens) -- NOT
      gpsimd partition_broadcast, whose nonzero output base silently
      miswrites on HW -- then one reciprocal_approx_fast and two DVE
      multiplies -> ot[pc] bf16.
  Wo: out rows st -> sum over pc of ot[pc].T @ Wo chunk, DVE copy, DMA
      out (tail chunks split across the sync+scalar DMA queues).

Measured: 332us HW exec, rel err 3.8e-3 (baseline 440us).
"""

import sys
from collections import deque

for _p in ("/opt/trn_rl_repo", "/root/.axon_site/_ro/trn_rl_repo"):
    if _p not in sys.path:
        sys.path.insert(0, _p)

import ml_dtypes
import numpy as np

import concourse.bass as bass
import concourse.mybir as mybir
import concourse.tile as tile
from concourse import bacc
from concourse.bass_utils import run_bass_kernel_spmd

F32 = mybir.dt.float32
BF16 = mybir.dt.bfloat16
EXPF = mybir.ActivationFunctionType.Exp

B, S, D, H = 4, 2048, 1024, 16
HD = D // H           # 64
HL = H // 2           # 8 heads per core
DL = HL * HD          # 512 local proj width
NEG = -30000.0        # causal mask additive value (exp underflows to 0)

N_DC = D // 128       # 8  d-chunks (projection contraction)
N_PC = HL // 2        # 4  head pairs
N_Q = S // 512        # 4  q-chunks (= stages)
N_ST = S // 128       # 16 s-tiles (output rows / V rows)
N_CC = D // 512       # 2  out column chunks


def build_program():
    s, d = S, D

    nc = bacc.Bacc("TRN2", target_bir_lowering=False, debug=False)

    # X arrives pre-transposed from the host ([d, s]): the on-chip DMA
    # XBAR transpose is a single serialized unit (~40us for 4MB); plain
    # row loads of X^T stream at full DMA bandwidth instead.
    X = nc.dram_tensor("X", [d, s], BF16, kind="ExternalInput")
    WQ = nc.dram_tensor("WQ", [d, DL], BF16, kind="ExternalInput")
    WK = nc.dram_tensor("WK", [d, DL], BF16, kind="ExternalInput")
    WV = nc.dram_tensor("WV", [d, DL], BF16, kind="ExternalInput")
    WO = nc.dram_tensor("WO", [DL, d], BF16, kind="ExternalInput")
    OUT = nc.dram_tensor("OUT", [s, d], F32, kind="ExternalOutput")

    with tile.TileContext(nc) as tc:
        with tc.tile_pool(name="persist", bufs=1) as persist:
            qt = [persist.tile([128, s], BF16, name=f"qt{i}") for i in range(N_PC)]
            kt = [persist.tile([128, s], BF16, name=f"kt{i}") for i in range(N_PC)]
            ot = [persist.tile([128, s], BF16, name=f"ot{i}") for i in range(N_PC)]
            xt = [persist.tile([128, s], BF16, name=f"xt{i}") for i in range(N_DC)]
            # AV stationary operand, per s-tile, per head block of 128 cols:
            #   even head (h0): cols 0:64 = V, col 64 = ones (denominator row)
            #   odd head (h1):  col 0 = ones, cols 64:128 = V
            # (the remaining cols feed PSUM rows nothing ever reads; all
            # consumed row ranges start at partition 0 or 64 — the DVE/BIR
            # partition-alignment rule)
            vt = [persist.tile([128, HL, 128], BF16, name=f"vt{i}")
                  for i in range(N_ST)]
            wq = persist.tile([128, N_DC, DL], BF16, name="wq")
            wk = persist.tile([128, N_DC, DL], BF16, name="wk")
            wv = persist.tile([128, N_DC, DL], BF16, name="wv")
            wo = persist.tile([128, N_PC, d], BF16, name="wo")

            # ---- input DMAs. The ramp is HBM-bandwidth-bound (~9MB of
            # inputs at ~360GB/s/core), so order by criticality: X^T rows
            # and Wq/Wk gate the first attention groups; Wv is needed a few
            # us later, Wo only tens of us in. ----
            nc.scalar.dma_start(
                wq[:], WQ.ap().rearrange("(c p) m -> p c m", p=128))
            for dc in range(0, 4):
                nc.sync.dma_start(
                    xt[dc][:], X[dc * 128:(dc + 1) * 128, :])
            nc.scalar.dma_start(
                wk[:], WK.ap().rearrange("(c p) m -> p c m", p=128))
            for dc in range(4, 8):
                nc.scalar.dma_start(
                    xt[dc][:], X[dc * 128:(dc + 1) * 128, :])
            nc.sync.dma_start(
                wv[:], WV.ap().rearrange("(c p) m -> p c m", p=128))
            nc.scalar.dma_start(
                wo[:], WO.ap().rearrange("(c p) m -> p c m", p=128))

            # ---- ramp: Q^T/K^T for q-chunk 0, accumulated per d-chunk so
            # matmuls start as soon as each X^T chunk lands (all Q chains
            # first: K is gated by the later wk DMA) ----
            with tc.tile_pool(name="ramp", bufs=1, space="PSUM") as ramp:
                accs = {}
                for wi in range(2):
                    for pc in range(N_PC):
                        accs[wi, pc] = ramp.tile(
                            [128, 512], F32, tag=f"acc{wi}_{pc}",
                            name=f"acc{wi}_{pc}")
                # HAM warmup: ~5us of dummy matmuls with no DMA dependencies
                # so the PE clock gate is at 8/8 (2.4GHz) when the real
                # chains arrive; the ramp otherwise runs at half clock
                nc.vector.memset(qt[0][:, 0:512], 0.0)
                for _ in range(12):
                    nc.tensor.matmul(
                        accs[0, 0][:], qt[0][:, 0:128], qt[0][:, 0:512],
                        start=True, stop=True)
                for wi, wsb in ((0, wq), (1, wk)):
                    for dc in range(N_DC):
                        for pc in range(N_PC):
                            nc.tensor.matmul(
                                accs[wi, pc][:],
                                wsb[:, dc, pc * 128:(pc + 1) * 128],
                                xt[dc][:, 0:512],
                                start=(dc == 0), stop=(dc == N_DC - 1))
                for wi, dst in ((0, qt), (1, kt)):
                    for pc in range(N_PC):
                        nc.scalar.copy(
                            dst[pc][:, 0:512], accs[wi, pc][:])

            # ones columns for the AV denominator rows (gpsimd, overlaps
            # the ramp DMAs/matmuls; only needed once AV starts)
            for st in range(N_ST):
                for g in range(N_PC):
                    nc.gpsimd.memset(vt[st][:, 2 * g, 64:65], 1.0)
                    nc.gpsimd.memset(vt[st][:, 2 * g + 1, 0:1], 1.0)
            # denominator-broadcast selector: rsel[c, p] = 1 iff p//64 == c,
            # so rsel.T @ [den0; den1] fills PSUM rows 0:64 with den0 and
            # 64:128 with den1
            rsel = persist.tile([2, 128], BF16)
            nc.gpsimd.memset(rsel[:], 1.0)
            nc.gpsimd.affine_select(
                out=rsel[:], in_=rsel[:],
                compare_op=mybir.AluOpType.is_ge, fill=0.0,
                base=0, pattern=[[1, 128]], channel_multiplier=-64,
            )
            nc.gpsimd.affine_select(
                out=rsel[:], in_=rsel[:],
                compare_op=mybir.AluOpType.is_ge, fill=0.0,
                base=63, pattern=[[-1, 128]], channel_multiplier=64,
            )


            with (
                tc.tile_pool(name="spp", bufs=2, space="PSUM") as spp,
                tc.tile_pool(name="avp", bufs=1, space="PSUM") as avp,
                tc.tile_pool(name="outp", bufs=2, space="PSUM") as outp,
                tc.tile_pool(name="etp", bufs=3) as etp,
                tc.tile_pool(name="nrm", bufs=2) as nrm,
            ):
                fillers = deque()
                uid = [0]

                def nid():
                    uid[0] += 1
                    return uid[0]

                def qk_chain_ops(wsb, dst, pc, nq):
                    box = {}
                    ops = []
                    for dc in range(N_DC):
                        def mm(dc=dc, box=box, wsb=wsb, pc=pc, nq=nq):
                            if dc == 0:
                                box["ps"] = outp.tile(
                                    [128, 512], F32, tag="outp",
                                    name=f"qkps{nid()}")
                            nc.tensor.matmul(
                                box["ps"][:],
                                wsb[:, dc, pc * 128:(pc + 1) * 128],
                                xt[dc][:, nq * 512:(nq + 1) * 512],
                                start=(dc == 0), stop=(dc == N_DC - 1))
                        ops.append(mm)

                    def cp(box=box, dst=dst, pc=pc, nq=nq):
                        # scalar engine: idle in the projection-heavy stages,
                        # and these copies gate the next stage's exps anyway
                        nc.scalar.copy(
                            dst[pc][:, nq * 512:(nq + 1) * 512], box["ps"][:])
                    ops.append(cp)
                    return ops

                def v_chain_ops(st):
                    box = {}
                    ops = []
                    for dc in range(N_DC):
                        def mm(dc=dc, box=box, st=st):
                            if dc == 0:
                                box["ps"] = outp.tile(
                                    [128, 512], F32, tag="outp",
                                    name=f"vps{nid()}")
                            nc.tensor.matmul(
                                box["ps"][:],
                                xt[dc][:, st * 128:(st + 1) * 128],
                                wv[:, dc, :],
                                start=(dc == 0), stop=(dc == N_DC - 1))
                        ops.append(mm)

                    def cp(box=box, st=st):
                        ps4 = box["ps"][:].rearrange(
                            "p (g t e) -> p g t e", t=2, e=64)
                        vtr = vt[st].rearrange("p (g t) c -> p g t c", t=2)
                        nc.scalar.copy(
                            vtr[:, :, 0:1, 0:64], ps4[:, :, 0:1, :])
                        nc.scalar.copy(
                            vtr[:, :, 1:2, 64:128], ps4[:, :, 1:2, :])
                    ops.append(cp)
                    return ops

                def wo_chain_ops(st, cc, tail=False):
                    box = {}
                    ops = []
                    for pc in range(N_PC):
                        def mm(pc=pc, box=box, st=st, cc=cc):
                            if pc == 0:
                                box["ps"] = outp.tile(
                                    [128, 512], F32, tag="outp",
                                    name=f"wops{nid()}")
                            nc.tensor.matmul(
                                box["ps"][:],
                                ot[pc][:, st * 128:(st + 1) * 128],
                                wo[:, pc, cc * 512:(cc + 1) * 512],
                                start=(pc == 0), stop=(pc == N_PC - 1))
                        ops.append(mm)

                    def fin(box=box, st=st, cc=cc, tail=tail):
                        osb = nrm.tile(
                            [128, 512], F32, tag="osb", name=f"osb{nid()}")
                        nc.vector.tensor_copy(osb[:], box["ps"][:])
                        # past the last exp the scalar hwdge queue is free
                        eng = nc.scalar if (tail and cc == 1) else nc.sync
                        eng.dma_start(
                            OUT[st * 128:(st + 1) * 128,
                                cc * 512:(cc + 1) * 512], osb[:])
                    ops.append(fin)
                    return ops

                def normalize(j, pc, av):
                    js = slice(j * 512, (j + 1) * 512)
                    orwA = nrm.tile([128, 512], F32, tag="orwA",
                                    name=f"orwA{nid()}")
                    orwB = nrm.tile([128, 512], F32, tag="orwB",
                                    name=f"orwB{nid()}")
                    # full-tile copies: rows 65+ of av0 / 1..63 of av1 are
                    # never read, copying them is cheaper than extra ops
                    nc.vector.tensor_copy(orwA[:], av[0][:])
                    nc.vector.tensor_copy(orwB[:], av[1][:])
                    dd = nrm.tile([2, 512], F32, tag="dd", name=f"dd{nid()}")
                    # the very last normalize runs after the final exp: use
                    # the then-idle scalar hwdge queue, dodging the sync
                    # queue's output-DMA backlog
                    deng = nc.scalar if (j == 3 and pc == 3) else nc.sync
                    deng.dma_start(dd[0:1, :], orwA[64:65, :])
                    deng.dma_start(dd[1:2, :], orwB[0:1, :])
                    ddb = nrm.tile([2, 512], BF16, tag="ddb",
                                   name=f"ddb{nid()}")
                    nc.vector.tensor_copy(ddb[:], dd[:])
                    bcp = outp.tile([128, 512], F32, tag="outp",
                                    name=f"bcp{nid()}")
                    nc.tensor.matmul(bcp[:], rsel[:], ddb[:],
                                     start=True, stop=True)
                    bcr = nrm.tile([128, 512], F32, tag="bcr",
                                   name=f"bcr{nid()}")
                    nc.vector.reciprocal_approx_fast(bcr[:], bcp[:])
                    nc.vector.tensor_mul(
                        ot[pc][0:64, js], orwA[0:64, :], bcr[0:64, :])
                    nc.vector.tensor_mul(
                        ot[pc][64:128, js], orwB[64:128, :], bcr[64:128, :])

                def attention_stage(j):
                    n_i = 4 * j + 4
                    # filler work to interleave into this stage: projections
                    # for the next stage; ALL Wo chains deferred to stage 3,
                    # whose exp stream is long enough to hide them
                    if j + 1 < N_Q:
                        for wsb, dst in ((wq, qt), (wk, kt)):
                            for pc in range(N_PC):
                                fillers.extend(
                                    qk_chain_ops(wsb, dst, pc, j + 1))
                        for st in range(4 * (j + 1), 4 * (j + 1) + 4):
                            fillers.extend(v_chain_ops(st))
                    if j == 3:
                        for st in range(0, 12):
                            for cc in range(N_CC):
                                fillers.extend(wo_chain_ops(st, cc))
                    n_groups = N_PC * n_i
                    pace = max(1, -(-len(fillers) // n_groups))

                    for pc in range(N_PC):
                        av = [avp.tile([128, 512], F32, tag=f"av{h}",
                                       name=f"av{j}_{pc}_{h}")
                              for h in (0, 1)]

                        def issue_av(i, rs, et, pc=pc, av=av, n_i=n_i):
                            for h in (0, 1):
                                nc.tensor.matmul(
                                    av[h][:, rs:512],
                                    vt[i][:, 2 * pc + h, :],
                                    et[:, 512 * h + rs:512 * h + 512],
                                    start=(i == 0), stop=(i == n_i - 1))

                        prev = None
                        for i in range(n_i):
                            r = i - 4 * j
                            rs = max(r, 0) * 128
                            stp = spp.tile([128, 1024], F32, tag="stp",
                                           name=f"stp{j}_{pc}_{i}")
                            for h in (0, 1):
                                hs = slice(64 * h, 64 * h + 64)
                                nc.tensor.matmul(
                                    stp[:, 512 * h + rs:512 * h + 512],
                                    kt[pc][hs, i * 128:(i + 1) * 128],
                                    qt[pc][hs, j * 512 + rs:(j + 1) * 512],
                                    start=True, stop=True,
                                    tile_position=(64 * h, 0))
                            if prev is not None:
                                issue_av(*prev)
                            et = etp.tile([128, 1024], BF16, tag="et",
                                          name=f"et{j}_{pc}_{i}")
                            if rs >= 256:
                                # mostly-masked diagonal group: two narrow
                                # exps beat one wide one (352-cycle fixed
                                # cost per ACTIVATE)
                                for h in (0, 1):
                                    sl = slice(512 * h + rs, 512 * h + 512)
                                    nc.scalar.activation(
                                        et[:, sl], stp[:, sl], EXPF,
                                        scale=0.125)
                            else:
                                nc.scalar.activation(
                                    et[:], stp[:], EXPF, scale=0.125)
                            if r >= 0:
                                # causal mask: zero the strict upper triangle
                                # of the diagonal 128-block (keep q >= k)
                                for h in (0, 1):
                                    sl = slice(512 * h + rs, 512 * h + rs + 128)
                                    nc.gpsimd.affine_select(
                                        out=et[:, sl], in_=et[:, sl],
                                        compare_op=mybir.AluOpType.is_ge,
                                        fill=0.0, base=0, pattern=[[1, 128]],
                                        channel_multiplier=-1)
                            prev = (i, rs, et)
                            # hold fillers near the pc boundary so the
                            # normalize's DVE/DMA chain isn't queued behind
                            # them (the next pc's AV reuses the av banks)
                            if i < n_i - 2:
                                for _ in range(pace):
                                    if fillers:
                                        fillers.popleft()()
                        issue_av(*prev)
                        normalize(j, pc, av)
                        for _ in range(2 * pace):
                            if fillers:
                                fillers.popleft()()
                    while fillers:
                        fillers.popleft()()

                # ---- V rows for stage 0, then the staged attention ----
                for st in range(4):
                    for op in v_chain_ops(st):
                        op()
                for j in range(N_Q):
                    attention_stage(j)
                # ---- tail: output rows of the last stage ----
                for st in range(12, 16):
                    for cc in range(N_CC):
                        for op in wo_chain_ops(st, cc, tail=True):
                            op()

    nc.compile()
    return nc


_NC_CACHE = {}


def _get_program():
    key = (S, D, HL)
    if key not in _NC_CACHE:
        _NC_CACHE[key] = build_program()
    return _NC_CACHE[key]


def _bf16(a):
    return np.ascontiguousarray(a.astype(ml_dtypes.bfloat16))


def make_in_maps(X, Wq, Wk, Wv, Wo):
    in_maps = []
    for c in range(8):
        b, hg = c // 2, c % 2
        cs = slice(hg * DL, hg * DL + DL)
        in_maps.append({
            "X": _bf16(X[b].T),
            "WQ": _bf16(Wq[:, cs]),
            "WK": _bf16(Wk[:, cs]),
            "WV": _bf16(Wv[:, cs]),
            "WO": _bf16(Wo[cs, :]),
        })
    return in_maps


def gather_out(results):
    out = np.empty((B, S, D), dtype=np.float32)
    for b in range(B):
        out[b] = results[2 * b]["OUT"] + results[2 * b + 1]["OUT"]
    return out


def kernel(X, Wq, Wk, Wv, Wo):
    X = np.asarray(X, dtype=np.float32)
    Wq = np.asarray(Wq, dtype=np.float32)
    Wk = np.asarray(Wk, dtype=np.float32)
    Wv = np.asarray(Wv, dtype=np.float32)
    Wo = np.asarray(Wo, dtype=np.float32)

    nc = _get_program()
    in_maps = make_in_maps(X, Wq, Wk, Wv, Wo)
    res = run_bass_kernel_spmd(nc, in_maps, list(range(8)), trace=False)
    return gather_out(res.results)


if __name__ == "__main__":
    rng = np.random.default_rng(0)
    scale = 1.0 / np.sqrt(D)
    inputs = {
        "X": rng.standard_normal((B, S, D), dtype=np.float32),
        "Wq": rng.standard_normal((D, D), dtype=np.float32) * scale,
        "Wk": rng.standard_normal((D, D), dtype=np.float32) * scale,
        "Wv": rng.standard_normal((D, D), dtype=np.float32) * scale,
        "Wo": rng.standard_normal((D, D), dtype=np.float32) * scale,
    }
    out = kernel(**inputs)
    print("kernel output shape:", out.shape)


# revision 40
# speedup vs baseline: 1.0077x; 1.0024x over previous
"""Trainium2 Bass kernel for multi-head causal self-attention.

Problem: X [4, 2048, 1024] fp32, Wq/Wk/Wv/Wo [1024, 1024], H=16 heads, HD=64.
reference: out = softmax_causal((X@Wq) (X@Wk)^T / 8) (X@Wv) merged @ Wo.

Sharding over 8 NeuronCores: core c handles batch b = c // 2 and head group
hg = c % 2 (8 heads each). Each core computes a partial [2048, 1024] output
(its heads' contribution through Wo's row shard); the host sums the two
partials per batch (the tensor-parallel all-reduce, done during unsharding).

Per-core dataflow (bf16 operands, fp32 PSUM accumulation), software-pipelined
so the scalar engine's exp stream starts early and runs continuously while
the PE fills gaps with projection / output-projection matmuls:

  ramp:   X arrives pre-transposed from the host (the on-chip DMA XBAR
          transpose is a serialized ~40us unit); DMAs ordered by
          criticality (X^T rows + Wq + Wk gate the first attention
          groups; the ramp is HBM-bandwidth-bound at ~360GB/s/core).
          Q^T/K^T for q-chunk 0 accumulate per-d-chunk as X^T rows land
          (8 PSUM banks), V rows 0..511 right after.
  stage j (= q-chunk): attention for q-chunk j; interleaved filler work =
          Q^T/K^T chunk j+1, V rows for stage j+1; all Wo chains are
          deferred to stage 3 whose long exp stream (ACT-bound) hides
          them. Fillers pause near each pc boundary so the normalize
          chain is not queued behind them on the DVE.
  group (j, pc, i): one 128-wide k-chunk for one head pair pc:
      S^T pair [128k, 2x512q] -> one 2-bank PSUM group (heads row-packed
      in the PE at tile_position 64h, streaming concurrently)
      one exp ACTIVATE over the [128, 1024] group -> et bf16 (batching
      amortizes ACT's 352-cycle per-instruction overhead)
      causal diagonal: post-exp gpsimd affine_select zeroes the upper
      triangle of the diagonal 128-block in et (SBUF; gpsimd cannot
      touch PSUM). Fully-masked leading columns are simply never read.
      AV: av_h += vt_h.T @ et_h; vt col layout gives h0 output at PSUM
      partitions 0..63 + denominator row 64, h1 denominator at row 0 +
      output partitions 64..127 (all row ranges 0/64-based: engine APs
      need 32-aligned partition bases).
  normalize (j, pc): DVE-copy av -> SBUF, gather the two denominator
      rows to partitions 0..1 via two small DMAs, cast bf16, broadcast
      to [128,512] with one PE selector matmul (rsel.T @ dens) -- NOT
      gpsimd partition_broadcast, whose nonzero output base silently
      miswrites on HW -- then one reciprocal_approx_fast and two DVE
      multiplies -> ot[pc] bf16.
  Wo: out rows st -> sum over pc of ot[pc].T @ Wo chunk, DVE copy, DMA
      out (tail chunks split across the sync+scalar DMA queues).

Measured: 332us HW exec, rel err 3.8e-3 (baseline 440us).
"""

import sys
from collections import deque

for _p in ("/opt/trn_rl_repo", "/root/.axon_site/_ro/trn_rl_repo"):
    if _p not in sys.path:
        sys.path.insert(0, _p)

import ml_dtypes
import numpy as np

import concourse.bass as bass
import concourse.mybir as mybir
import concourse.tile as tile
from concourse import bacc
from concourse.bass_utils import run_bass_kernel_spmd

F32 = mybir.dt.float32
BF16 = mybir.dt.bfloat16
EXPF = mybir.ActivationFunctionType.Exp

B, S, D, H = 4, 2048, 1024, 16
HD = D // H           # 64
HL = H // 2           # 8 heads per core
DL = HL * HD          # 512 local proj width
NEG = -30000.0        # causal mask additive value (exp underflows to 0)

N_DC = D // 128       # 8  d-chunks (projection contraction)
N_PC = HL // 2        # 4  head pairs
N_Q = S // 512        # 4  q-chunks (= stages)
N_ST = S // 128       # 16 s-tiles (output rows / V rows)
N_CC = D // 512       # 2  out column chunks


def build_program():
    s, d = S, D

    nc = bacc.Bacc("TRN2", target_bir_lowering=False, debug=False)

    # X arrives pre-transposed from the host ([d, s]): the on-chip DMA
    # XBAR transpose is a single serialized unit (~40us for 4MB); plain
    # row loads of X^T stream at full DMA bandwidth instead.
    X = nc.dram_tensor("X", [d, s], BF16, kind="ExternalInput")
    WQ = nc.dram_tensor("WQ", [d, DL], BF16, kind="ExternalInput")
    WK = nc.dram_tensor("WK", [d, DL], BF16, kind="ExternalInput")
    WV = nc.dram_tensor("WV", [d, DL], BF16, kind="ExternalInput")
    WO = nc.dram_tensor("WO", [DL, d], BF16, kind="ExternalInput")
    OUT = nc.dram_tensor("OUT", [s, d], F32, kind="ExternalOutput")

    with tile.TileContext(nc) as tc:
        with tc.tile_pool(name="persist", bufs=1) as persist:
            qt = [persist.tile([128, s], BF16, name=f"qt{i}") for i in range(N_PC)]
            kt = [persist.tile([128, s], BF16, name=f"kt{i}") for i in range(N_PC)]
            ot = [persist.tile([128, s], BF16, name=f"ot{i}") for i in range(N_PC)]
            xt = [persist.tile([128, s], BF16, name=f"xt{i}") for i in range(N_DC)]
            # AV stationary operand, per s-tile, per head block of 128 cols:
            #   even head (h0): cols 0:64 = V, col 64 = ones (denominator row)
            #   odd head (h1):  col 0 = ones, cols 64:128 = V
            # (the remaining cols feed PSUM rows nothing ever reads; all
            # consumed row ranges start at partition 0 or 64 — the DVE/BIR
            # partition-alignment rule)
            vt = [persist.tile([128, HL, 128], BF16, name=f"vt{i}")
                  for i in range(N_ST)]
            wq = persist.tile([128, N_DC, DL], BF16, name="wq")
            wk = persist.tile([128, N_DC, DL], BF16, name="wk")
            wv = persist.tile([128, N_DC, DL], BF16, name="wv")
            wo = persist.tile([128, N_PC, d], BF16, name="wo")

            # ---- input DMAs. The ramp is HBM-bandwidth-bound (~9MB of
            # inputs at ~360GB/s/core), so order by criticality: X^T rows
            # and Wq/Wk gate the first attention groups; Wv is needed a few
            # us later, Wo only tens of us in. ----
            nc.scalar.dma_start(
                wq[:], WQ.ap().rearrange("(c p) m -> p c m", p=128))
            for dc in range(0, 4):
                nc.sync.dma_start(
                    xt[dc][:], X[dc * 128:(dc + 1) * 128, :])
            nc.scalar.dma_start(
                wk[:], WK.ap().rearrange("(c p) m -> p c m", p=128))
            for dc in range(4, 8):
                nc.scalar.dma_start(
                    xt[dc][:], X[dc * 128:(dc + 1) * 128, :])
            nc.sync.dma_start(
                wv[:], WV.ap().rearrange("(c p) m -> p c m", p=128))
            nc.scalar.dma_start(
                wo[:], WO.ap().rearrange("(c p) m -> p c m", p=128))

            # ---- ramp: Q^T/K^T for q-chunk 0, accumulated per d-chunk so
            # matmuls start as soon as each X^T chunk lands (all Q chains
            # first: K is gated by the later wk DMA) ----
            with tc.tile_pool(name="ramp", bufs=1, space="PSUM") as ramp:
                accs = {}
                for wi in range(2):
                    for pc in range(N_PC):
                        accs[wi, pc] = ramp.tile(
                            [128, 512], F32, tag=f"acc{wi}_{pc}",
                            name=f"acc{wi}_{pc}")
                # HAM warmup: ~5us of dummy matmuls with no DMA dependencies
                # so the PE clock gate is at 8/8 (2.4GHz) when the real
                # chains arrive; the ramp otherwise runs at half clock
                nc.vector.memset(qt[0][:, 0:512], 0.0)
                for _ in range(12):
                    nc.tensor.matmul(
                        accs[0, 0][:], qt[0][:, 0:128], qt[0][:, 0:512],
                        start=True, stop=True)
                for wi, wsb in ((0, wq), (1, wk)):
                    for dc in range(N_DC):
                        for pc in range(N_PC):
                            nc.tensor.matmul(
                                accs[wi, pc][:],
                                wsb[:, dc, pc * 128:(pc + 1) * 128],
                                xt[dc][:, 0:512],
                                start=(dc == 0), stop=(dc == N_DC - 1))
                for wi, dst in ((0, qt), (1, kt)):
                    for pc in range(N_PC):
                        nc.scalar.copy(
                            dst[pc][:, 0:512], accs[wi, pc][:])

            # ones columns for the AV denominator rows (gpsimd, overlaps
            # the ramp DMAs/matmuls; only needed once AV starts)
            for st in range(N_ST):
                for g in range(N_PC):
                    nc.gpsimd.memset(vt[st][:, 2 * g, 64:65], 1.0)
                    nc.gpsimd.memset(vt[st][:, 2 * g + 1, 0:1], 1.0)
            # denominator-broadcast selector: rsel[c, p] = 1 iff p//64 == c,
            # so rsel.T @ [den0; den1] fills PSUM rows 0:64 with den0 and
            # 64:128 with den1
            rsel = persist.tile([2, 128], BF16)
            nc.gpsimd.memset(rsel[:], 1.0)
            nc.gpsimd.affine_select(
                out=rsel[:], in_=rsel[:],
                compare_op=mybir.AluOpType.is_ge, fill=0.0,
                base=0, pattern=[[1, 128]], channel_multiplier=-64,
            )
            nc.gpsimd.affine_select(
                out=rsel[:], in_=rsel[:],
                compare_op=mybir.AluOpType.is_ge, fill=0.0,
                base=63, pattern=[[-1, 128]], channel_multiplier=64,
            )


            with (
                tc.tile_pool(name="spp", bufs=2, space="PSUM") as spp,
                tc.tile_pool(name="avp", bufs=1, space="PSUM") as avp,
                tc.tile_pool(name="outp", bufs=2, space="PSUM") as outp,
                tc.tile_pool(name="etp", bufs=3) as etp,
                tc.tile_pool(name="nrm", bufs=2) as nrm,
            ):
                fillers = deque()
                uid = [0]

                def nid():
                    uid[0] += 1
                    return uid[0]

                def qk_chain_ops(wsb, dst, pc, nq):
                    box = {}
                    ops = []
                    for dc in range(N_DC):
                        def mm(dc=dc, box=box, wsb=wsb, pc=pc, nq=nq):
                            if dc == 0:
                                box["ps"] = outp.tile(
                                    [128, 512], F32, tag="outp",
                                    name=f"qkps{nid()}")
                            nc.tensor.matmul(
                                box["ps"][:],
                                wsb[:, dc, pc * 128:(pc + 1) * 128],
                                xt[dc][:, nq * 512:(nq + 1) * 512],
                                start=(dc == 0), stop=(dc == N_DC - 1))
                        ops.append(mm)

                    def cp(box=box, dst=dst, pc=pc, nq=nq):
                        # scalar engine: idle in the projection-heavy stages,
                        # and these copies gate the next stage's exps anyway
                        nc.scalar.copy(
                            dst[pc][:, nq * 512:(nq + 1) * 512], box["ps"][:])
                    ops.append(cp)
                    return ops

                def v_chain_ops(st):
                    box = {}
                    ops = []
                    for dc in range(N_DC):
                        def mm(dc=dc, box=box, st=st):
                            if dc == 0:
                                box["ps"] = outp.tile(
                                    [128, 512], F32, tag="outp",
                                    name=f"vps{nid()}")
                            nc.tensor.matmul(
                                box["ps"][:],
                                xt[dc][:, st * 128:(st + 1) * 128],
                                wv[:, dc, :],
                                start=(dc == 0), stop=(dc == N_DC - 1))
                        ops.append(mm)

                    def cp(box=box, st=st):
                        ps4 = box["ps"][:].rearrange(
                            "p (g t e) -> p g t e", t=2, e=64)
                        vtr = vt[st].rearrange("p (g t) c -> p g t c", t=2)
                        nc.scalar.copy(
                            vtr[:, :, 0:1, 0:64], ps4[:, :, 0:1, :])
                        nc.scalar.copy(
                            vtr[:, :, 1:2, 64:128], ps4[:, :, 1:2, :])
                    ops.append(cp)
                    return ops

                def wo_chain_ops(st, cc, tail=False):
                    box = {}
                    ops = []
                    for pc in range(N_PC):
                        def mm(pc=pc, box=box, st=st, cc=cc):
                            if pc == 0:
                                box["ps"] = outp.tile(
                                    [128, 512], F32, tag="outp",
                                    name=f"wops{nid()}")
                            nc.tensor.matmul(
                                box["ps"][:],
                                ot[pc][:, st * 128:(st + 1) * 128],
                                wo[:, pc, cc * 512:(cc + 1) * 512],
                                start=(pc == 0), stop=(pc == N_PC - 1))
                        ops.append(mm)

                    def fin(box=box, st=st, cc=cc, tail=tail):
                        osb = nrm.tile(
                            [128, 512], F32, tag="osb", name=f"osb{nid()}")
                        nc.vector.tensor_copy(osb[:], box["ps"][:])
                        # past the last exp the scalar hwdge queue is free
                        eng = nc.scalar if (tail and cc == 1) else nc.sync
                        eng.dma_start(
                            OUT[st * 128:(st + 1) * 128,
                                cc * 512:(cc + 1) * 512], osb[:])
                    ops.append(fin)
                    return ops

                wo3_store = {}

                def wo3_partial_ops(st, cc):
                    # stage-3 output rows: accumulate head pairs 0..2 and
                    # stage to SBUF while pair 3's attention/normalize is
                    # still in flight; the tail adds pair 3's term only
                    box = {}
                    ops = []
                    for pc in range(3):
                        def mm(pc=pc, box=box, st=st, cc=cc):
                            if pc == 0:
                                box["ps"] = outp.tile(
                                    [128, 512], F32, tag="outp",
                                    name=f"w3p{nid()}")
                            nc.tensor.matmul(
                                box["ps"][:],
                                ot[pc][:, st * 128:(st + 1) * 128],
                                wo[:, pc, cc * 512:(cc + 1) * 512],
                                start=(pc == 0), stop=(pc == 2))
                        ops.append(mm)

                    def cp(box=box, st=st, cc=cc):
                        osb = nrm.tile([128, 512], F32, tag="osb3", bufs=8,
                                       name=f"osb3_{nid()}")
                        nc.vector.tensor_copy(osb[:], box["ps"][:])
                        wo3_store[(st, cc)] = osb
                    ops.append(cp)
                    return ops

                def normalize(j, pc, av):
                    js = slice(j * 512, (j + 1) * 512)
                    orwA = nrm.tile([128, 512], F32, tag="orwA",
                                    name=f"orwA{nid()}")
                    orwB = nrm.tile([128, 512], F32, tag="orwB",
                                    name=f"orwB{nid()}")
                    # full-tile copies: rows 65+ of av0 / 1..63 of av1 are
                    # never read, copying them is cheaper than extra ops
                    nc.vector.tensor_copy(orwA[:], av[0][:])
                    nc.vector.tensor_copy(orwB[:], av[1][:])
                    dd = nrm.tile([2, 512], F32, tag="dd", name=f"dd{nid()}")
                    # the very last normalize runs after the final exp: use
                    # the then-idle scalar hwdge queue, dodging the sync
                    # queue's output-DMA backlog
                    deng = nc.scalar if (j == 3 and pc == 3) else nc.sync
                    deng.dma_start(dd[0:1, :], orwA[64:65, :])
                    deng.dma_start(dd[1:2, :], orwB[0:1, :])
                    ddb = nrm.tile([2, 512], BF16, tag="ddb",
                                   name=f"ddb{nid()}")
                    nc.vector.tensor_copy(ddb[:], dd[:])
                    bcp = outp.tile([128, 512], F32, tag="outp",
                                    name=f"bcp{nid()}")
                    nc.tensor.matmul(bcp[:], rsel[:], ddb[:],
                                     start=True, stop=True)
                    bcr = nrm.tile([128, 512], F32, tag="bcr",
                                   name=f"bcr{nid()}")
                    nc.vector.reciprocal_approx_fast(bcr[:], bcp[:])
                    nc.vector.tensor_mul(
                        ot[pc][0:64, js], orwA[0:64, :], bcr[0:64, :])
                    nc.vector.tensor_mul(
                        ot[pc][64:128, js], orwB[64:128, :], bcr[64:128, :])

                def attention_stage(j):
                    n_i = 4 * j + 4
                    # filler work to interleave into this stage: projections
                    # for the next stage; ALL Wo chains deferred to stage 3,
                    # whose exp stream is long enough to hide them
                    if j + 1 < N_Q:
                        for wsb, dst in ((wq, qt), (wk, kt)):
                            for pc in range(N_PC):
                                fillers.extend(
                                    qk_chain_ops(wsb, dst, pc, j + 1))
                        for st in range(4 * (j + 1), 4 * (j + 1) + 4):
                            fillers.extend(v_chain_ops(st))
                    if j == 3:
                        for st in range(0, 12):
                            for cc in range(N_CC):
                                fillers.extend(wo_chain_ops(st, cc))
                    n_groups = N_PC * n_i
                    pace = max(1, -(-len(fillers) // n_groups))

                    for pc in range(N_PC):
                        av = [avp.tile([128, 512], F32, tag=f"av{h}",
                                       name=f"av{j}_{pc}_{h}")
                              for h in (0, 1)]

                        def issue_av(i, rs, et, pc=pc, av=av, n_i=n_i):
                            for h in (0, 1):
                                nc.tensor.matmul(
                                    av[h][:, rs:512],
                                    vt[i][:, 2 * pc + h, :],
                                    et[:, 512 * h + rs:512 * h + 512],
                                    start=(i == 0), stop=(i == n_i - 1))

                        prev = None
                        for i in range(n_i):
                            r = i - 4 * j
                            rs = max(r, 0) * 128
                            stp = spp.tile([128, 1024], F32, tag="stp",
                                           name=f"stp{j}_{pc}_{i}")
                            for h in (0, 1):
                                hs = slice(64 * h, 64 * h + 64)
                                nc.tensor.matmul(
                                    stp[:, 512 * h + rs:512 * h + 512],
                                    kt[pc][hs, i * 128:(i + 1) * 128],
                                    qt[pc][hs, j * 512 + rs:(j + 1) * 512],
                                    start=True, stop=True,
                                    tile_position=(64 * h, 0))
                            if prev is not None:
                                issue_av(*prev)
                            et = etp.tile([128, 1024], BF16, tag="et",
                                          name=f"et{j}_{pc}_{i}")
                            if rs >= 256:
                                # mostly-masked diagonal group: two narrow
                                # exps beat one wide one (352-cycle fixed
                                # cost per ACTIVATE)
                                for h in (0, 1):
                                    sl = slice(512 * h + rs, 512 * h + 512)
                                    nc.scalar.activation(
                                        et[:, sl], stp[:, sl], EXPF,
                                        scale=0.125)
                            else:
                                nc.scalar.activation(
                                    et[:], stp[:], EXPF, scale=0.125)
                            if r >= 0:
                                # causal mask: zero the strict upper triangle
                                # of the diagonal 128-block (keep q >= k)
                                for h in (0, 1):
                                    sl = slice(512 * h + rs, 512 * h + rs + 128)
                                    nc.gpsimd.affine_select(
                                        out=et[:, sl], in_=et[:, sl],
                                        compare_op=mybir.AluOpType.is_ge,
                                        fill=0.0, base=0, pattern=[[1, 128]],
                                        channel_multiplier=-1)
                            prev = (i, rs, et)
                            # hold fillers near the pc boundary so the
                            # normalize's DVE/DMA chain isn't queued behind
                            # them (the next pc's AV reuses the av banks)
                            if i < n_i - 2:
                                for _ in range(pace):
                                    if fillers:
                                        fillers.popleft()()
                        issue_av(*prev)
                        normalize(j, pc, av)
                        if j == 3 and pc == 2:
                            for st in range(12, 16):
                                for cc in range(N_CC):
                                    fillers.extend(wo3_partial_ops(st, cc))
                        for _ in range(2 * pace):
                            if fillers:
                                fillers.popleft()()
                    while fillers:
                        fillers.popleft()()

                # ---- V rows for stage 0, then the staged attention ----
                for st in range(4):
                    for op in v_chain_ops(st):
                        op()
                for j in range(N_Q):
                    attention_stage(j)
                # ---- tail: add head pair 3's term to the staged partials
                for st in range(12, 16):
                    for cc in range(N_CC):
                        psb = outp.tile([128, 512], F32, tag="outp",
                                        name=f"w3f{nid()}")
                        nc.tensor.matmul(
                            psb[:], ot[3][:, st * 128:(st + 1) * 128],
                            wo[:, 3, cc * 512:(cc + 1) * 512],
                            start=True, stop=True)
                        osb = wo3_store[(st, cc)]
                        nc.vector.tensor_add(osb[:], osb[:], psb[:])
                        eng = nc.scalar if cc == 1 else nc.sync
                        eng.dma_start(
                            OUT[st * 128:(st + 1) * 128,
                                cc * 512:(cc + 1) * 512], osb[:])

    nc.compile()
    return nc


_NC_CACHE = {}


def _get_program():
    key = (S, D, HL)
    if key not in _NC_CACHE:
        _NC_CACHE[key] = build_program()
    return _NC_CACHE[key]


def _bf16(a):
    return np.ascontiguousarray(a.astype(ml_dtypes.bfloat16))


def make_in_maps(X, Wq, Wk, Wv, Wo):
    in_maps = []
    for c in range(8):
        b, hg = c // 2, c % 2
        cs = slice(hg * DL, hg * DL + DL)
        in_maps.append({
            "X": _bf16(X[b].T),
            "WQ": _bf16(Wq[:, cs]),
            "WK": _bf16(Wk[:, cs]),
            "WV": _bf16(Wv[:, cs]),
            "WO": _bf16(Wo[cs, :]),
        })
    return in_maps


def gather_out(results):
    out = np.empty((B, S, D), dtype=np.float32)
    for b in range(B):
        out[b] = results[2 * b]["OUT"] + results[2 * b + 1]["OUT"]
    return out


def kernel(X, Wq, Wk, Wv, Wo):
    X = np.asarray(X, dtype=np.float32)
    Wq = np.asarray(Wq, dtype=np.float32)
    Wk = np.asarray(Wk, dtype=np.float32)
    Wv = np.asarray(Wv, dtype=np.float32)
    Wo = np.asarray(Wo, dtype=np.float32)

    nc = _get_program()
    in_maps = make_in_maps(X, Wq, Wk, Wv, Wo)
    res = run_bass_kernel_spmd(nc, in_maps, list(range(8)), trace=False)
    return gather_out(res.results)


if __name__ == "__main__":
    rng = np.random.default_rng(0)
    scale = 1.0 / np.sqrt(D)
    inputs = {
        "X": rng.standard_normal((B, S, D), dtype=np.float32),
        "Wq": rng.standard_normal((D, D), dtype=np.float32) * scale,
        "Wk": rng.standard_normal((D, D), dtype=np.float32) * scale,
        "Wv": rng.standard_normal((D, D), dtype=np.float32) * scale,
        "Wo": rng.standard_normal((D, D), dtype=np.float32) * scale,
    }
    out = kernel(**inputs)
    print("kernel output shape:", out.shape)


# revision 44
# speedup vs baseline: 1.0123x; 1.0046x over previous
"""Trainium2 Bass kernel for multi-head causal self-attention.

Problem: X [4, 2048, 1024] fp32, Wq/Wk/Wv/Wo [1024, 1024], H=16 heads, HD=64.
reference: out = softmax_causal((X@Wq) (X@Wk)^T / 8) (X@Wv) merged @ Wo.

Sharding over 8 NeuronCores: core c handles batch b = c // 2 and head group
hg = c % 2 (8 heads each). Each core computes a partial [2048, 1024] output
(its heads' contribution through Wo's row shard); the host sums the two
partials per batch (the tensor-parallel all-reduce, done during unsharding).

Per-core dataflow (bf16 operands, fp32 PSUM accumulation), software-pipelined
so the scalar engine's exp stream starts early and runs continuously while
the PE fills gaps with projection / output-projection matmuls:

  ramp:   X arrives pre-transposed from the host (the on-chip DMA XBAR
          transpose is a serialized ~40us unit); DMAs ordered by
          criticality (X^T rows + Wq + Wk gate the first attention
          groups; the ramp is HBM-bandwidth-bound at ~360GB/s/core).
          Q^T/K^T for q-chunk 0 accumulate per-d-chunk as X^T rows land
          (8 PSUM banks), V rows 0..511 right after.
  stage j (= q-chunk): attention for q-chunk j; interleaved filler work =
          Q^T/K^T chunk j+1, V rows for stage j+1; all Wo chains are
          deferred to stage 3 whose long exp stream (ACT-bound) hides
          them. Fillers pause near each pc boundary so the normalize
          chain is not queued behind them on the DVE.
  group (j, pc, i): one 128-wide k-chunk for one head pair pc:
      S^T pair [128k, 2x512q] -> one 2-bank PSUM group (heads row-packed
      in the PE at tile_position 64h, streaming concurrently)
      one exp ACTIVATE over the [128, 1024] group -> et bf16 (batching
      amortizes ACT's 352-cycle per-instruction overhead)
      causal diagonal: post-exp gpsimd affine_select zeroes the upper
      triangle of the diagonal 128-block in et (SBUF; gpsimd cannot
      touch PSUM). Fully-masked leading columns are simply never read.
      AV: av_h += vt_h.T @ et_h; vt col layout gives h0 output at PSUM
      partitions 0..63 + denominator row 64, h1 denominator at row 0 +
      output partitions 64..127 (all row ranges 0/64-based: engine APs
      need 32-aligned partition bases).
  normalize (j, pc): DVE-copy av -> SBUF, gather the two denominator
      rows to partitions 0..1 via two small DMAs, cast bf16, broadcast
      to [128,512] with one PE selector matmul (rsel.T @ dens) -- NOT
      gpsimd partition_broadcast, whose nonzero output base silently
      miswrites on HW -- then one reciprocal_approx_fast and two DVE
      multiplies -> ot[pc] bf16.
  Wo: out rows st -> sum over pc of ot[pc].T @ Wo chunk, DVE copy, DMA
      out (tail chunks split across the sync+scalar DMA queues).

Measured: 332us HW exec, rel err 3.8e-3 (baseline 440us).
"""

import sys
from collections import deque

for _p in ("/opt/trn_rl_repo", "/root/.axon_site/_ro/trn_rl_repo"):
    if _p not in sys.path:
        sys.path.insert(0, _p)

import ml_dtypes
import numpy as np

import concourse.bass as bass
import concourse.mybir as mybir
import concourse.tile as tile
from concourse import bacc
from concourse.bass_utils import run_bass_kernel_spmd

F32 = mybir.dt.float32
BF16 = mybir.dt.bfloat16
EXPF = mybir.ActivationFunctionType.Exp

B, S, D, H = 4, 2048, 1024, 16
HD = D // H           # 64
HL = H // 2           # 8 heads per core
DL = HL * HD          # 512 local proj width
NEG = -30000.0        # causal mask additive value (exp underflows to 0)

N_DC = D // 128       # 8  d-chunks (projection contraction)
N_PC = HL // 2        # 4  head pairs
N_Q = S // 512        # 4  q-chunks (= stages)
N_ST = S // 128       # 16 s-tiles (output rows / V rows)
N_CC = D // 512       # 2  out column chunks


def build_program():
    s, d = S, D

    nc = bacc.Bacc("TRN2", target_bir_lowering=False, debug=False)

    # X arrives pre-transposed from the host ([d, s]): the on-chip DMA
    # XBAR transpose is a single serialized unit (~40us for 4MB); plain
    # row loads of X^T stream at full DMA bandwidth instead.
    X = nc.dram_tensor("X", [d, s], BF16, kind="ExternalInput")
    WQ = nc.dram_tensor("WQ", [d, DL], BF16, kind="ExternalInput")
    WK = nc.dram_tensor("WK", [d, DL], BF16, kind="ExternalInput")
    WV = nc.dram_tensor("WV", [d, DL], BF16, kind="ExternalInput")
    WO = nc.dram_tensor("WO", [DL, d], BF16, kind="ExternalInput")
    # bf16 output halves the 8MB store traffic on the sync DMA queue; the
    # host sums the two per-batch partials in fp32
    OUT = nc.dram_tensor("OUT", [s, d], BF16, kind="ExternalOutput")

    with tile.TileContext(nc) as tc:
        with tc.tile_pool(name="persist", bufs=1) as persist:
            qt = [persist.tile([128, s], BF16, name=f"qt{i}") for i in range(N_PC)]
            kt = [persist.tile([128, s], BF16, name=f"kt{i}") for i in range(N_PC)]
            ot = [persist.tile([128, s], BF16, name=f"ot{i}") for i in range(N_PC)]
            xt = [persist.tile([128, s], BF16, name=f"xt{i}") for i in range(N_DC)]
            # AV stationary operand, per s-tile, per head block of 128 cols:
            #   even head (h0): cols 0:64 = V, col 64 = ones (denominator row)
            #   odd head (h1):  col 0 = ones, cols 64:128 = V
            # (the remaining cols feed PSUM rows nothing ever reads; all
            # consumed row ranges start at partition 0 or 64 — the DVE/BIR
            # partition-alignment rule)
            vt = [persist.tile([128, HL, 128], BF16, name=f"vt{i}")
                  for i in range(N_ST)]
            wq = persist.tile([128, N_DC, DL], BF16, name="wq")
            wk = persist.tile([128, N_DC, DL], BF16, name="wk")
            wv = persist.tile([128, N_DC, DL], BF16, name="wv")
            wo = persist.tile([128, N_PC, d], BF16, name="wo")

            # ---- input DMAs. The ramp is HBM-bandwidth-bound (~9MB of
            # inputs at ~360GB/s/core), so order by criticality: X^T rows
            # and Wq/Wk gate the first attention groups; Wv is needed a few
            # us later, Wo only tens of us in. ----
            nc.scalar.dma_start(
                wq[:], WQ.ap().rearrange("(c p) m -> p c m", p=128))
            for dc in range(0, 4):
                nc.sync.dma_start(
                    xt[dc][:], X[dc * 128:(dc + 1) * 128, :])
            nc.scalar.dma_start(
                wk[:], WK.ap().rearrange("(c p) m -> p c m", p=128))
            for dc in range(4, 8):
                nc.scalar.dma_start(
                    xt[dc][:], X[dc * 128:(dc + 1) * 128, :])
            nc.sync.dma_start(
                wv[:], WV.ap().rearrange("(c p) m -> p c m", p=128))
            nc.scalar.dma_start(
                wo[:], WO.ap().rearrange("(c p) m -> p c m", p=128))

            # ---- ramp: Q^T/K^T for q-chunk 0, accumulated per d-chunk so
            # matmuls start as soon as each X^T chunk lands (all Q chains
            # first: K is gated by the later wk DMA) ----
            with tc.tile_pool(name="ramp", bufs=1, space="PSUM") as ramp:
                accs = {}
                for wi in range(2):
                    for pc in range(N_PC):
                        accs[wi, pc] = ramp.tile(
                            [128, 512], F32, tag=f"acc{wi}_{pc}",
                            name=f"acc{wi}_{pc}")
                # HAM warmup: ~5us of dummy matmuls with no DMA dependencies
                # so the PE clock gate is at 8/8 (2.4GHz) when the real
                # chains arrive; the ramp otherwise runs at half clock
                nc.vector.memset(qt[0][:, 0:512], 0.0)
                for _ in range(12):
                    nc.tensor.matmul(
                        accs[0, 0][:], qt[0][:, 0:128], qt[0][:, 0:512],
                        start=True, stop=True)
                for wi, wsb in ((0, wq), (1, wk)):
                    for dc in range(N_DC):
                        for pc in range(N_PC):
                            nc.tensor.matmul(
                                accs[wi, pc][:],
                                wsb[:, dc, pc * 128:(pc + 1) * 128],
                                xt[dc][:, 0:512],
                                start=(dc == 0), stop=(dc == N_DC - 1))
                for wi, dst in ((0, qt), (1, kt)):
                    for pc in range(N_PC):
                        nc.scalar.copy(
                            dst[pc][:, 0:512], accs[wi, pc][:])

            # ones columns for the AV denominator rows (gpsimd, overlaps
            # the ramp DMAs/matmuls; only needed once AV starts)
            for st in range(N_ST):
                for g in range(N_PC):
                    nc.gpsimd.memset(vt[st][:, 2 * g, 64:65], 1.0)
                    nc.gpsimd.memset(vt[st][:, 2 * g + 1, 0:1], 1.0)
            # denominator-broadcast selector: rsel[c, p] = 1 iff p//64 == c,
            # so rsel.T @ [den0; den1] fills PSUM rows 0:64 with den0 and
            # 64:128 with den1
            rsel = persist.tile([2, 128], BF16)
            nc.gpsimd.memset(rsel[:], 1.0)
            nc.gpsimd.affine_select(
                out=rsel[:], in_=rsel[:],
                compare_op=mybir.AluOpType.is_ge, fill=0.0,
                base=0, pattern=[[1, 128]], channel_multiplier=-64,
            )
            nc.gpsimd.affine_select(
                out=rsel[:], in_=rsel[:],
                compare_op=mybir.AluOpType.is_ge, fill=0.0,
                base=63, pattern=[[-1, 128]], channel_multiplier=64,
            )


            with (
                tc.tile_pool(name="spp", bufs=2, space="PSUM") as spp,
                tc.tile_pool(name="avp", bufs=1, space="PSUM") as avp,
                tc.tile_pool(name="outp", bufs=2, space="PSUM") as outp,
                tc.tile_pool(name="etp", bufs=3) as etp,
                tc.tile_pool(name="nrm", bufs=2) as nrm,
            ):
                fillers = deque()
                uid = [0]

                def nid():
                    uid[0] += 1
                    return uid[0]

                def qk_chain_ops(wsb, dst, pc, nq):
                    box = {}
                    ops = []
                    for dc in range(N_DC):
                        def mm(dc=dc, box=box, wsb=wsb, pc=pc, nq=nq):
                            if dc == 0:
                                box["ps"] = outp.tile(
                                    [128, 512], F32, tag="outp",
                                    name=f"qkps{nid()}")
                            nc.tensor.matmul(
                                box["ps"][:],
                                wsb[:, dc, pc * 128:(pc + 1) * 128],
                                xt[dc][:, nq * 512:(nq + 1) * 512],
                                start=(dc == 0), stop=(dc == N_DC - 1))
                        ops.append(mm)

                    def cp(box=box, dst=dst, pc=pc, nq=nq):
                        # scalar engine: idle in the projection-heavy stages,
                        # and these copies gate the next stage's exps anyway
                        nc.scalar.copy(
                            dst[pc][:, nq * 512:(nq + 1) * 512], box["ps"][:])
                    ops.append(cp)
                    return ops

                def v_chain_ops(st):
                    box = {}
                    ops = []
                    for dc in range(N_DC):
                        def mm(dc=dc, box=box, st=st):
                            if dc == 0:
                                box["ps"] = outp.tile(
                                    [128, 512], F32, tag="outp",
                                    name=f"vps{nid()}")
                            nc.tensor.matmul(
                                box["ps"][:],
                                xt[dc][:, st * 128:(st + 1) * 128],
                                wv[:, dc, :],
                                start=(dc == 0), stop=(dc == N_DC - 1))
                        ops.append(mm)

                    def cp(box=box, st=st):
                        ps4 = box["ps"][:].rearrange(
                            "p (g t e) -> p g t e", t=2, e=64)
                        vtr = vt[st].rearrange("p (g t) c -> p g t c", t=2)
                        nc.scalar.copy(
                            vtr[:, :, 0:1, 0:64], ps4[:, :, 0:1, :])
                        nc.scalar.copy(
                            vtr[:, :, 1:2, 64:128], ps4[:, :, 1:2, :])
                    ops.append(cp)
                    return ops

                def wo_chain_ops(st, cc, tail=False):
                    box = {}
                    ops = []
                    for pc in range(N_PC):
                        def mm(pc=pc, box=box, st=st, cc=cc):
                            if pc == 0:
                                box["ps"] = outp.tile(
                                    [128, 512], F32, tag="outp",
                                    name=f"wops{nid()}")
                            nc.tensor.matmul(
                                box["ps"][:],
                                ot[pc][:, st * 128:(st + 1) * 128],
                                wo[:, pc, cc * 512:(cc + 1) * 512],
                                start=(pc == 0), stop=(pc == N_PC - 1))
                        ops.append(mm)

                    def fin(box=box, st=st, cc=cc, tail=tail):
                        osb = nrm.tile(
                            [128, 512], BF16, tag="osb", name=f"osb{nid()}")
                        nc.vector.tensor_copy(osb[:], box["ps"][:])
                        # past the last exp the scalar hwdge queue is free
                        eng = nc.scalar if (tail and cc == 1) else nc.sync
                        eng.dma_start(
                            OUT[st * 128:(st + 1) * 128,
                                cc * 512:(cc + 1) * 512], osb[:])
                    ops.append(fin)
                    return ops

                wo3_store = {}

                def wo3_partial_ops(st, cc):
                    # stage-3 output rows: accumulate head pairs 0..2 and
                    # stage to SBUF while pair 3's attention/normalize is
                    # still in flight; the tail adds pair 3's term only
                    box = {}
                    ops = []
                    for pc in range(3):
                        def mm(pc=pc, box=box, st=st, cc=cc):
                            if pc == 0:
                                box["ps"] = outp.tile(
                                    [128, 512], F32, tag="outp",
                                    name=f"w3p{nid()}")
                            nc.tensor.matmul(
                                box["ps"][:],
                                ot[pc][:, st * 128:(st + 1) * 128],
                                wo[:, pc, cc * 512:(cc + 1) * 512],
                                start=(pc == 0), stop=(pc == 2))
                        ops.append(mm)

                    def cp(box=box, st=st, cc=cc):
                        osb = nrm.tile([128, 512], BF16, tag="osb3", bufs=8,
                                       name=f"osb3_{nid()}")
                        nc.vector.tensor_copy(osb[:], box["ps"][:])
                        wo3_store[(st, cc)] = osb
                    ops.append(cp)
                    return ops

                def normalize(j, pc, av):
                    js = slice(j * 512, (j + 1) * 512)
                    orwA = nrm.tile([128, 512], F32, tag="orwA",
                                    name=f"orwA{nid()}")
                    orwB = nrm.tile([128, 512], F32, tag="orwB",
                                    name=f"orwB{nid()}")
                    # full-tile copies: rows 65+ of av0 / 1..63 of av1 are
                    # never read, copying them is cheaper than extra ops
                    nc.vector.tensor_copy(orwA[:], av[0][:])
                    nc.vector.tensor_copy(orwB[:], av[1][:])
                    dd = nrm.tile([2, 512], F32, tag="dd", name=f"dd{nid()}")
                    # the very last normalize runs after the final exp: use
                    # the then-idle scalar hwdge queue, dodging the sync
                    # queue's output-DMA backlog
                    deng = nc.scalar if (j == 3 and pc == 3) else nc.sync
                    deng.dma_start(dd[0:1, :], orwA[64:65, :])
                    deng.dma_start(dd[1:2, :], orwB[0:1, :])
                    ddb = nrm.tile([2, 512], BF16, tag="ddb",
                                   name=f"ddb{nid()}")
                    nc.vector.tensor_copy(ddb[:], dd[:])
                    bcp = outp.tile([128, 512], F32, tag="outp",
                                    name=f"bcp{nid()}")
                    nc.tensor.matmul(bcp[:], rsel[:], ddb[:],
                                     start=True, stop=True)
                    bcr = nrm.tile([128, 512], F32, tag="bcr",
                                   name=f"bcr{nid()}")
                    nc.vector.reciprocal_approx_fast(bcr[:], bcp[:])
                    nc.vector.tensor_mul(
                        ot[pc][0:64, js], orwA[0:64, :], bcr[0:64, :])
                    nc.vector.tensor_mul(
                        ot[pc][64:128, js], orwB[64:128, :], bcr[64:128, :])

                def attention_stage(j):
                    n_i = 4 * j + 4
                    # filler work to interleave into this stage: projections
                    # for the next stage; ALL Wo chains deferred to stage 3,
                    # whose exp stream is long enough to hide them
                    if j + 1 < N_Q:
                        for wsb, dst in ((wq, qt), (wk, kt)):
                            for pc in range(N_PC):
                                fillers.extend(
                                    qk_chain_ops(wsb, dst, pc, j + 1))
                        for st in range(4 * (j + 1), 4 * (j + 1) + 4):
                            fillers.extend(v_chain_ops(st))
                    if j == 3:
                        for st in range(0, 12):
                            for cc in range(N_CC):
                                fillers.extend(wo_chain_ops(st, cc))
                    n_groups = N_PC * n_i
                    pace = max(1, -(-len(fillers) // n_groups))

                    for pc in range(N_PC):
                        av = [avp.tile([128, 512], F32, tag=f"av{h}",
                                       name=f"av{j}_{pc}_{h}")
                              for h in (0, 1)]

                        def issue_av(i, rs, et, pc=pc, av=av, n_i=n_i):
                            for h in (0, 1):
                                nc.tensor.matmul(
                                    av[h][:, rs:512],
                                    vt[i][:, 2 * pc + h, :],
                                    et[:, 512 * h + rs:512 * h + 512],
                                    start=(i == 0), stop=(i == n_i - 1))

                        prev = None
                        for i in range(n_i):
                            r = i - 4 * j
                            rs = max(r, 0) * 128
                            stp = spp.tile([128, 1024], F32, tag="stp",
                                           name=f"stp{j}_{pc}_{i}")
                            for h in (0, 1):
                                hs = slice(64 * h, 64 * h + 64)
                                nc.tensor.matmul(
                                    stp[:, 512 * h + rs:512 * h + 512],
                                    kt[pc][hs, i * 128:(i + 1) * 128],
                                    qt[pc][hs, j * 512 + rs:(j + 1) * 512],
                                    start=True, stop=True,
                                    tile_position=(64 * h, 0))
                            if prev is not None:
                                issue_av(*prev)
                            et = etp.tile([128, 1024], BF16, tag="et",
                                          name=f"et{j}_{pc}_{i}")
                            if rs >= 256:
                                # mostly-masked diagonal group: two narrow
                                # exps beat one wide one (352-cycle fixed
                                # cost per ACTIVATE)
                                for h in (0, 1):
                                    sl = slice(512 * h + rs, 512 * h + 512)
                                    nc.scalar.activation(
                                        et[:, sl], stp[:, sl], EXPF,
                                        scale=0.125)
                            else:
                                nc.scalar.activation(
                                    et[:], stp[:], EXPF, scale=0.125)
                            if r >= 0:
                                # causal mask: zero the strict upper triangle
                                # of the diagonal 128-block (keep q >= k)
                                for h in (0, 1):
                                    sl = slice(512 * h + rs, 512 * h + rs + 128)
                                    nc.gpsimd.affine_select(
                                        out=et[:, sl], in_=et[:, sl],
                                        compare_op=mybir.AluOpType.is_ge,
                                        fill=0.0, base=0, pattern=[[1, 128]],
                                        channel_multiplier=-1)
                            prev = (i, rs, et)
                            # hold fillers near the pc boundary so the
                            # normalize's DVE/DMA chain isn't queued behind
                            # them (the next pc's AV reuses the av banks)
                            if i < n_i - 2:
                                for _ in range(pace):
                                    if fillers:
                                        fillers.popleft()()
                        issue_av(*prev)
                        normalize(j, pc, av)
                        if j == 3 and pc == 2:
                            for st in range(12, 16):
                                for cc in range(N_CC):
                                    fillers.extend(wo3_partial_ops(st, cc))
                        for _ in range(2 * pace):
                            if fillers:
                                fillers.popleft()()
                    while fillers:
                        fillers.popleft()()

                # ---- V rows for stage 0, then the staged attention ----
                for st in range(4):
                    for op in v_chain_ops(st):
                        op()
                for j in range(N_Q):
                    attention_stage(j)
                # ---- tail: add head pair 3's term to the staged partials
                for st in range(12, 16):
                    for cc in range(N_CC):
                        psb = outp.tile([128, 512], F32, tag="outp",
                                        name=f"w3f{nid()}")
                        nc.tensor.matmul(
                            psb[:], ot[3][:, st * 128:(st + 1) * 128],
                            wo[:, 3, cc * 512:(cc + 1) * 512],
                            start=True, stop=True)
                        osb = wo3_store[(st, cc)]
                        nc.vector.tensor_add(osb[:], osb[:], psb[:])
                        eng = nc.scalar if cc == 1 else nc.sync
                        eng.dma_start(
                            OUT[st * 128:(st + 1) * 128,
                                cc * 512:(cc + 1) * 512], osb[:])

    nc.compile()
    return nc


_NC_CACHE = {}


def _get_program():
    key = (S, D, HL)
    if key not in _NC_CACHE:
        _NC_CACHE[key] = build_program()
    return _NC_CACHE[key]


def _bf16(a):
    return np.ascontiguousarray(a.astype(ml_dtypes.bfloat16))


def make_in_maps(X, Wq, Wk, Wv, Wo):
    in_maps = []
    for c in range(8):
        b, hg = c // 2, c % 2
        cs = slice(hg * DL, hg * DL + DL)
        in_maps.append({
            "X": _bf16(X[b].T),
            "WQ": _bf16(Wq[:, cs]),
            "WK": _bf16(Wk[:, cs]),
            "WV": _bf16(Wv[:, cs]),
            "WO": _bf16(Wo[cs, :]),
        })
    return in_maps


def gather_out(results):
    out = np.empty((B, S, D), dtype=np.float32)
    for b in range(B):
        out[b] = (results[2 * b]["OUT"].astype(np.float32)
                  + results[2 * b + 1]["OUT"].astype(np.float32))
    return out


def kernel(X, Wq, Wk, Wv, Wo):
    X = np.asarray(X, dtype=np.float32)
    Wq = np.asarray(Wq, dtype=np.float32)
    Wk = np.asarray(Wk, dtype=np.float32)
    Wv = np.asarray(Wv, dtype=np.float32)
    Wo = np.asarray(Wo, dtype=np.float32)

    nc = _get_program()
    in_maps = make_in_maps(X, Wq, Wk, Wv, Wo)
    res = run_bass_kernel_spmd(nc, in_maps, list(range(8)), trace=False)
    return gather_out(res.results)


if __name__ == "__main__":
    rng = np.random.default_rng(0)
    scale = 1.0 / np.sqrt(D)
    inputs = {
        "X": rng.standard_normal((B, S, D), dtype=np.float32),
        "Wq": rng.standard_normal((D, D), dtype=np.float32) * scale,
        "Wk": rng.standard_normal((D, D), dtype=np.float32) * scale,
        "Wv": rng.standard_normal((D, D), dtype=np.float32) * scale,
        "Wo": rng.standard_normal((D, D), dtype=np.float32) * scale,
    }
    out = kernel(**inputs)
    print("kernel output shape:", out.shape)
